# revision 17
# baseline (speedup 1.0000x reference)
"""GAU (gated attention unit, relu^2 linear attention) Trainium2 kernel, v2.

Sharding: 8 cores = batch (4) x T-half (2).  Each core handles 2048 rows
of both the query and key/value streams of one batch.  The kv/k_sum
reduction over keys is completed with a 2-rank AllReduce between the two
cores of each batch (bf16 payload); everything else is fully local.

v2 structure (vs v1): the value projection is folded to AFTER the key
reduction -- kv_raw[e,s] = sum_k values[k,e]*kfeat[k,s] is accumulated
against RAW values (token-major), allreduced, then kv = Wv @ kv_raw +
bv x k_sum is a tiny [1024x512 @ 1024x1024] fold done per-core.  This
removes the per-token v projection (Tk*D*D MACs).  k_sum rides along as
an extra matmul with an all-ones stationary vector.  All biases (bk,
bq, bv) are folded into matmul contraction rows via padded chunks; the
key mask is folded into the key operand on the host; the query mask and
output bias bo are applied on the host after the gather.  relu^2 is a
single vector stt (max(x,0)*x).  Matmuls are fp8(e4m3) DoubleRow
throughout; PSUM accumulation fp32.  Output is bf16, upcast on host.

Scales: kvr_sb = kv_raw/32, kv_sb = kv/32, ks_sb = k_sum/8,
uq = 0.5*u*(qkv/32) -> zi = qm*64/(8*zp + EPS).
"""
import sys

sys.path.insert(0, "/opt/trn_rl_repo")

import ml_dtypes
import numpy as np
import concourse.bass as bass
import concourse.mybir as mybir
import concourse.tile as tile
from concourse.bass_utils import run_bass_kernel_spmd

AF = mybir.ActivationFunctionType
ALU = mybir.AluOpType
PM = mybir.MatmulPerfMode
F32 = mybir.dt.float32
BF16 = mybir.dt.bfloat16
F8 = mybir.dt.float8e4
NPBF = ml_dtypes.bfloat16
NPF8 = mybir.dt.np(F8)

N_CORES = 8
D = 1024
S = 512
EPS = 1e-6
KVR = 32.0   # kvr_sb/kv_sb hold kv_raw/32, kv/32
KS = 8.0     # ks_sb holds k_sum/8
UQ = 0.5     # uq holds 0.5 * u * (qkv/32)


def split_sync_waits(nc, max_waits=1):
    """The pinned walrus accepts at most one sync wait per instruction;
    hoist excess waits onto same-engine NoOps inserted before the
    offending instruction (same engine => identical semantics)."""
    n = 0
    for bb in nc.main_func.blocks:
        out = []
        for inst in bb.instructions:
            si = inst.sync_info
            if si is not None and si.on_wait and len(si.on_wait) > max_waits:
                waits = list(si.on_wait)
                spill, keep = waits[:-max_waits], waits[-max_waits:]
                for j in range(0, len(spill), max_waits):
                    nop = mybir.InstNoOp(
                        name=f"{inst.name}_wsp{j}",
                        engine=inst.engine,
                        ins=[],
                        outs=[],
                        bass_nofuse=True,
                        sync_info=mybir.SyncInfo(
                            on_wait=spill[j : j + max_waits], on_update=[]
                        ),
                    )
                    nc.register_instruction(nop)
                    out.append(nop)
                    n += 1
                si.on_wait[:] = keep
            out.append(inst)
        bb.instructions[:] = out
    return n


def dedup_ldweights(nc):
    """Delete an InstLdweights whose weights AP is byte-identical to the
    previous PE ldweights with only matmuls in between (the PE array
    keeps its stationary operand across matmuls).  Waits/updates of the
    deleted LDW migrate to the next PE instruction (its paired matmul)
    so sync semantics are preserved.  Run BEFORE split_sync_waits."""
    n = 0
    for bb in nc.main_func.blocks:
        out = []
        last_sig = None
        pending = None  # sync carried from a deleted LDW
        for inst in bb.instructions:
            tn = type(inst).__name__
            if getattr(inst, "engine", None) != mybir.EngineType.PE:
                out.append(inst)
                continue
            if tn == "InstLdweights":
                ap = inst.ins[0]
                sig = (ap.memref, ap.offset, tuple(map(tuple, ap.ap)),
                       str(inst.perf_mode), str(inst.is_transpose),
                       str(inst.tile_position), str(inst.tile_size))
                if sig == last_sig:
                    si = inst.sync_info
                    if si is not None and (si.on_wait or si.on_update):
                        if pending is None:
                            pending = ([], [])
                        pending[0].extend(si.on_wait)
                        pending[1].extend(si.on_update)
                    n += 1
                    continue
                last_sig = sig
            elif tn == "InstMatmult":
                if inst.is_transpose:
                    last_sig = None
            else:
                # any other PE instruction: be conservative
                last_sig = None
            if pending is not None:
                si = inst.sync_info
                if si is None:
                    inst.sync_info = mybir.SyncInfo(
                        on_wait=list(pending[0]), on_update=list(pending[1]))
                else:
                    si.on_wait[:] = list(pending[0]) + list(si.on_wait)
                    si.on_update[:] = list(si.on_update) + list(pending[1])
                pending = None
            out.append(inst)
        assert pending is None
        bb.instructions[:] = out
    return n


def build_nc(T=2048, use_collective=True):
    NT = T // 128    # t-tiles
    NQ = T // 512    # q-chunks (phase 2)
    NDA = 5          # DoubleRow pairs over d (4 data + 1 bias/pad)
    NS = S // 128    # s-tiles
    NF = D // 128    # f-tiles (gate dim)

    nc = bass.Bass("TRN2", target_bir_lowering=False, debug=False,
                   num_devices=N_CORES)

    # ---- I/O ------------------------------------------------------------
    # host-prepped: chunked, fp8, masks/biases folded in (see make_in_maps)
    qTc = nc.dram_tensor("qTc", [NQ, 128, NDA * 2 * 512], F8, kind="ExternalInput")
    kTc = nc.dram_tensor("kTc", [NT, 128, NDA * 2 * 128], F8, kind="ExternalInput")
    vTc = nc.dram_tensor("vTc", [NT, 128, D + 128], F8, kind="ExternalInput")
    wkc = nc.dram_tensor("wkc", [128, NDA * 2 * S], F8, kind="ExternalInput")
    wqc = nc.dram_tensor("wqc", [128, NDA * 2 * S], F8, kind="ExternalInput")
    wgc = nc.dram_tensor("wgc", [128, (D // 128) * D], F8, kind="ExternalInput")
    wvc = nc.dram_tensor("wvc", [128, NDA * 2 * D], F8, kind="ExternalInput")
    woc = nc.dram_tensor("woc", [128, (D // 128) * D], F8, kind="ExternalInput")
    bgc = nc.dram_tensor("bgc", [128, NF], F32, kind="ExternalInput")
    qm = nc.dram_tensor("qm", [128, NT], F32, kind="ExternalInput")
    out = nc.dram_tensor("out", [T, D], BF16, kind="ExternalOutput")

    with tile.TileContext(nc) as tc:
        with tc.tile_pool(name="const", bufs=1) as cp:
            # first-needed first
            wk_sb = cp.tile([128, NDA * 2, S], F8)
            nc.sync.dma_start(wk_sb[:], wkc.ap())
            # persistent data
            v_all = cp.tile([128, NT, D + 128], F8)
            k_nat = cp.tile([128, NT, S], F8)
            q_sb = cp.tile([128, NS, T], F8)
            u_sb = cp.tile([128, NF, T], F8)
            uq_f8 = cp.tile([128, NF, T], F8)
            kvr_sb = cp.tile([128, NDA * 2, S], F8)
            kv_sb = cp.tile([128, NS, D], F8)
            ks_sb = cp.tile([128, NS, 1], F8)
            # deferred-load tiles
            wq_sb = cp.tile([128, NDA * 2, S], F8)
            wg_sb = cp.tile([128, D // 128, D], F8)
            wv_sb = cp.tile([128, NDA * 2, D], F8)
            wo_sb = cp.tile([128, D // 128, D], F8)
            qm_sb = cp.tile([128, NT], F32)
            bg_sb = cp.tile([128, NF], F32)
            eps_sb = cp.tile([128, NQ], F32)

            with tc.tile_pool(name="dram", bufs=1, space="DRAM") as dram, \
                 tc.tile_pool(name="pf", bufs=1) as pf:
                bounce_in = dram.tile([2 * S + 1, S], BF16)
                bounce_out = dram.tile([2 * S + 1, S], BF16)

                def load_qc(qch):
                    qc = pf.tile([128, NDA * 2, 512], F8, name="qc",
                                 tag="qc", bufs=4)
                    nc.sync.dma_start(qc[:], qTc.ap()[qch])
                    return qc
                qc_pre = {}

                # ============ phase 1: k feats, kv_raw e0..3, k_sum ======
                with tc.tile_pool(name="p1", bufs=1) as p1, \
                     tc.tile_pool(name="ps1", bufs=1, space="PSUM") as ps1:
                    kv04 = [ps1.tile([128, S], F32, name=f"kv0_{e}", tag="kv0",
                                     bufs=NS) for e in range(NS)]

                    for j in range(NT // 2):
                        for tt in (2 * j, 2 * j + 1):
                            kc = p1.tile([128, NDA * 2, 128], F8, name="kc",
                                         tag="kc", bufs=4)
                            nc.sync.dma_start(kc[:], kTc.ap()[tt])
                            nc.sync.dma_start(v_all[:, tt, :], vTc.ap()[tt])
                            if tt == 0:
                                # zero the bias/pad chunks of kvr (stale f8
                                # NaN x 0 would poison the fold PSUM)
                                nc.gpsimd.memset(kvr_sb[:, 8:10, :], 0.0)
                            if tt == 1:
                                qc_pre[0] = load_qc(0)
                            if tt in (3, 5, 7) and (tt - 1) // 2 < NQ:
                                qc_pre[(tt - 1) // 2] = load_qc((tt - 1) // 2)
                                nc.sync.dma_start(wq_sb[:], wqc.ap())
                                nc.sync.dma_start(qm_sb[:], qm.ap())
                                nc.sync.dma_start(bg_sb[:], bgc.ap())
                                nc.vector.memset(eps_sb[:], EPS)
                            if tt == 2:
                                nc.sync.dma_start(wg_sb[:], wgc.ap())
                            if tt == 3:
                                nc.sync.dma_start(wv_sb[:], wvc.ap())
                            if tt == 4:
                                nc.sync.dma_start(wo_sb[:], woc.ap())

                            # k feats: relu(K Wk^T + bk)^2 * km (bias+mask
                            # folded into padded chunks) -> k_nat[:,tt,:]
                            kb = ps1.tile([128, S], F32, name="kb", tag="kb",
                                          bufs=3)
                            for c in range(NDA):
                                nc.tensor.matmul(kb[:], kc[:, 2 * c:2 * c + 2, :],
                                                 wk_sb[:, 2 * c:2 * c + 2, :],
                                                 start=(c == 0), stop=(c == NDA - 1),
                                                 perf_mode=PM.DoubleRow)
                            krelu = p1.tile([128, S], BF16, name="krelu",
                                            tag="krelu", bufs=2)
                            nc.vector.tensor_scalar_max(krelu[:], kb[:], 0.0)
                            nc.scalar.activation(k_nat[:, tt, :], krelu[:],
                                                 AF.Square)

                        # kv_raw e-blocks 0..3 accumulate over t pairs
                        for e in range(NS):
                            nc.tensor.matmul(
                                kv04[e][:],
                                v_all[:, 2 * j:2 * j + 2, e * 128:(e + 1) * 128],
                                k_nat[:, 2 * j:2 * j + 2, :],
                                start=(j == 0), stop=(j == NT // 2 - 1),
                                perf_mode=PM.DoubleRow)

                    for e in range(NS):
                        kvst = p1.tile([128, S], BF16, name="kvst",
                                       tag="kvst", bufs=2)
                        nc.scalar.activation(kvst[:], kv04[e][:], AF.Copy)
                        nc.sync.dma_start(
                            bounce_in[e * 128:(e + 1) * 128, :], kvst[:])

                # ============ phase 1b: kv_raw e4..7 =====================
                with tc.tile_pool(name="p1b", bufs=1) as p1b, \
                     tc.tile_pool(name="ps1b", bufs=1, space="PSUM") as ps1b:
                    kv48 = [ps1b.tile([128, S], F32, name=f"kv1_{e}", tag="kv1",
                                      bufs=NS) for e in range(NS)]
                    ks_blk = ps1b.tile([128, S], F32, name="ks_blk",
                                       tag="ksb", bufs=1)
                    for j in range(NT // 2):
                        for e in range(NS):
                            nc.tensor.matmul(
                                kv48[e][:],
                                v_all[:, 2 * j:2 * j + 2,
                                      S + e * 128:S + (e + 1) * 128],
                                k_nat[:, 2 * j:2 * j + 2, :],
                                start=(j == 0), stop=(j == NT // 2 - 1),
                                perf_mode=PM.DoubleRow)
                        nc.tensor.matmul(
                            ks_blk[:],
                            v_all[:, 2 * j:2 * j + 2, D:D + 128],
                            k_nat[:, 2 * j:2 * j + 2, :],
                            start=(j == 0), stop=(j == NT // 2 - 1),
                            perf_mode=PM.DoubleRow)
                    for e in range(NS):
                        kvst1 = p1b.tile([128, S], BF16, name="kvst1",
                                         tag="kvst1", bufs=2)
                        nc.scalar.activation(kvst1[:], kv48[e][:], AF.Copy)
                        nc.sync.dma_start(
                            bounce_in[S + e * 128:S + (e + 1) * 128, :],
                            kvst1[:])
                    ksst = p1b.tile([1, S], BF16, name="ksst")
                    nc.scalar.activation(ksst[:], ks_blk[0:1, :], AF.Copy)
                    nc.sync.dma_start(bounce_in[2 * S:2 * S + 1, :], ksst[:])


                with tc.tile_pool(name="p2", bufs=1) as p2, \
                     tc.tile_pool(name="ps2a", bufs=1, space="PSUM") as ps2a:
                    if use_collective:
                        nc.gpsimd.collective_compute(
                            "AllReduce", ALU.add,
                            replica_groups=[[0, 1], [2, 3], [4, 5], [6, 7]],
                            ins=[bounce_in.opt()], outs=[bounce_out.opt()])
                        kv_src = bounce_out
                    else:
                        kv_src = bounce_in

                    # ---- pass A: q feats + u gate (no kv dependency) ----
                    # loops run stationary-major so one LDWEIGHTS serves
                    # all NQ moving chunks (dedup_ldweights strips the rest)
                    qcs = []
                    for qch in range(NQ):
                        qc = qc_pre.pop(qch, None)
                        qcs.append(qc if qc is not None else load_qc(qch))
                    for s in range(NS):
                        qfs = [ps2a.tile([128, 512], F32, name=f"qf{q}",
                                         tag="mm", bufs=8) for q in range(NQ)]
                        for c in range(NDA):
                            for qch in range(NQ):
                                nc.tensor.matmul(
                                    qfs[qch][:],
                                    wq_sb[:, 2 * c:2 * c + 2,
                                          s * 128:(s + 1) * 128],
                                    qcs[qch][:, 2 * c:2 * c + 2, :],
                                    start=(c == 0), stop=(c == NDA - 1),
                                    perf_mode=PM.DoubleRow)
                        for qch in range(NQ):
                            t0 = qch * 512
                            qrelu = p2.tile([128, 512], BF16, name="qrelu",
                                            tag="qrelu", bufs=4)
                            nc.vector.tensor_scalar_max(qrelu[:], qfs[qch][:],
                                                        0.0)
                            nc.gpsimd.tensor_mul(q_sb[:, s, t0:t0 + 512],
                                                 qrelu[:], qrelu[:])
                    for f in range(NF):
                        ufs = [ps2a.tile([128, 512], F32, name=f"uf{q}",
                                         tag="mm", bufs=8) for q in range(NQ)]
                        for c in range(NDA - 1):
                            for qch in range(NQ):
                                nc.tensor.matmul(
                                    ufs[qch][:],
                                    wg_sb[:, 2 * c:2 * c + 2,
                                          f * 128:(f + 1) * 128],
                                    qcs[qch][:, 2 * c:2 * c + 2, :],
                                    start=(c == 0), stop=(c == NDA - 2),
                                    perf_mode=PM.DoubleRow)
                        for qch in range(NQ):
                            t0 = qch * 512
                            nc.scalar.activation(u_sb[:, f, t0:t0 + 512],
                                                 ufs[qch][:], AF.Silu,
                                                 bias=bg_sb[:, f:f + 1])

                    # ---- unpack kv_raw + k_sum from the collective ------
                    kvf_sb = p2.tile([128, 2 * NS, S], BF16, name="kvf")
                    nc.sync.dma_start(
                        kvf_sb[:],
                        kv_src[0:2 * S, :].rearrange("(c p) s -> p c s", p=128))
                    ksf = p2.tile([1, S], BF16, name="ksf")
                    nc.sync.dma_start(ksf[:], kv_src[2 * S:2 * S + 1, :])
                    ks4 = p2.tile([128, NS], BF16, name="ks4")
                    nc.sync.dma_start(
                        ks4[:],
                        kv_src[2 * S:2 * S + 1, :].rearrange(
                            "o (c p) -> p (c o)", p=128))
                    for c in range(NS):
                        nc.scalar.activation(kvr_sb[:, 2 * c:2 * c + 2, :],
                                             kvf_sb[:, 2 * c:2 * c + 2, :],
                                             AF.Copy, scale=1.0 / KVR)
                    nc.scalar.activation(kvr_sb[0:1, 8, :], ksf[:], AF.Copy,
                                         scale=1.0 / KVR)
                    nc.vector.tensor_scalar_mul(ks_sb[:, :, 0], ks4[:],
                                                1.0 / KS)

                with tc.tile_pool(name="p2b", bufs=1) as p2b, \
                     tc.tile_pool(name="ps2b", bufs=1, space="PSUM") as ps2b:
                    # ---- fold kv = Wv @ kv_raw + bv x k_sum -------------
                    for sb in range(NS):
                        fps = [ps2b.tile([128, 512], F32, name=f"fp{h}",
                                         tag="mm", bufs=4) for h in range(2)]
                        for c in range(NDA):
                            for half in range(2):
                                nc.tensor.matmul(
                                    fps[half][:],
                                    kvr_sb[:, 2 * c:2 * c + 2,
                                           sb * 128:(sb + 1) * 128],
                                    wv_sb[:, 2 * c:2 * c + 2,
                                          half * 512:(half + 1) * 512],
                                    start=(c == 0), stop=(c == NDA - 1),
                                    perf_mode=PM.DoubleRow)
                        for half in range(2):
                            nc.vector.tensor_copy(
                                kv_sb[:, sb, half * 512:(half + 1) * 512],
                                fps[half][:])

                    # ---- pass B: qkv (f-major), z, output projection ----
                    for f in range(NF):
                        qks = [ps2b.tile([128, 512], F32, name=f"qk{q}",
                                         tag="mm", bufs=4) for q in range(NQ)]
                        for c in range(NS // 2):
                            for qch in range(NQ):
                                nc.tensor.matmul(
                                    qks[qch][:],
                                    kv_sb[:, 2 * c:2 * c + 2,
                                          f * 128:(f + 1) * 128],
                                    q_sb[:, 2 * c:2 * c + 2,
                                         qch * 512:(qch + 1) * 512],
                                    start=(c == 0), stop=(c == NS // 2 - 1),
                                    perf_mode=PM.DoubleRow)
                        for qch in range(NQ):
                            t0 = qch * 512
                            nc.vector.scalar_tensor_tensor(
                                uq_f8[:, f, t0:t0 + 512],
                                u_sb[:, f, t0:t0 + 512], UQ, qks[qch][:],
                                op0=ALU.mult, op1=ALU.mult)

                    for qch in range(NQ):
                        zp = ps2b.tile([128, 512], F32, name="zp", tag="mm",
                                       bufs=4)
                        for tt in range(4):
                            ti = qch * 4 + tt
                            for c in range(NS // 2):
                                nc.tensor.matmul(
                                    zp[:, tt:tt + 1],
                                    q_sb[:, 2 * c:2 * c + 2,
                                         ti * 128:(ti + 1) * 128],
                                    ks_sb[:, 2 * c:2 * c + 2, :],
                                    start=(c == 0), stop=(c == NS // 2 - 1),
                                    perf_mode=PM.DoubleRow)
                        z1 = p2b.tile([128, NQ], F32, name="z1", tag="z1",
                                      bufs=2)
                        nc.vector.scalar_tensor_tensor(
                            z1[:], zp[:, 0:NQ], KS, eps_sb[:], op0=ALU.mult,
                            op1=ALU.add)
                        z2 = p2b.tile([128, NQ], F32, name="z2", tag="z2",
                                      bufs=2)
                        nc.vector.reciprocal(z2[:], z1[:])
                        zi4 = p2b.tile([128, NQ], F32, name="zi4", tag="zi4",
                                       bufs=2)
                        nc.vector.scalar_tensor_tensor(
                            zi4[:], qm_sb[:, qch * 4:(qch + 1) * 4],
                            KVR / UQ, z2[:], op0=ALU.mult, op1=ALU.mult)

                        for tt in range(4):
                            ti = qch * 4 + tt
                            op = ps2b.tile([128, D], F32, name="op",
                                           tag="out", bufs=2)
                            for f2 in range(NF // 2):
                                for half in range(2):
                                    nc.tensor.matmul(
                                        op[:, half * 512:(half + 1) * 512],
                                        uq_f8[:, 2 * f2:2 * f2 + 2,
                                              ti * 128:(ti + 1) * 128],
                                        wo_sb[:, 2 * f2:2 * f2 + 2,
                                              half * 512:(half + 1) * 512],
                                        start=(f2 == 0),
                                        stop=(f2 == NF // 2 - 1),
                                        perf_mode=PM.DoubleRow)
                            o_sb = p2b.tile([128, D], BF16, name="o_sb",
                                            tag="o_sb", bufs=3)
                            nc.scalar.activation(o_sb[:], op[:], AF.Copy,
                                                 scale=zi4[:, tt:tt + 1])
                            nc.sync.dma_start(
                                out.ap()[ti * 128:(ti + 1) * 128, :], o_sb[:])

    dedup_ldweights(nc)
    split_sync_waits(nc)
    return nc


_NC_CACHE = {}


def _get_nc(T, use_collective=True):
    key = (T, use_collective)
    if key not in _NC_CACHE:
        _NC_CACHE[key] = build_nc(T, use_collective)
    return _NC_CACHE[key]


def _chunk_T_aug(xT, chunk, extra_row=None):
    """[D, T] -> [T//chunk, 128, 10*chunk] fp8: chunks 0..7 d-major data,
    chunk 8 row 0 = extra_row (per-token), chunk 9 = 0."""
    Dd, T = xT.shape
    nt = T // chunk
    out = np.zeros((nt, 128, 10 * chunk), dtype=NPF8)
    x = xT.reshape(Dd // 128, 128, nt, chunk).transpose(2, 1, 0, 3)
    out[:, :, : (Dd // 128) * chunk] = x.reshape(nt, 128, -1).astype(NPF8)
    if extra_row is not None:
        out[:, 0, 8 * chunk:9 * chunk] = extra_row.reshape(nt, chunk).astype(NPF8)
    return np.ascontiguousarray(out)


def _chunk_W_aug(wT, extra_row=None):
    """[D, F] -> [128, 10*F] fp8: chunks 0..7 d-major, chunk 8 row 0 =
    extra_row (bias over F), chunk 9 = 0."""
    Dd, Fd = wT.shape
    out = np.zeros((128, 10 * Fd), dtype=NPF8)
    w = wT.reshape(Dd // 128, 128, Fd).transpose(1, 0, 2).reshape(128, -1)
    out[:, : (Dd // 128) * Fd] = w.astype(NPF8)
    if extra_row is not None:
        out[0, 8 * Fd:9 * Fd] = np.asarray(extra_row, np.float32).astype(NPF8)
    return np.ascontiguousarray(out)


def _chunk_W(wT):
    Dd, Fd = wT.shape
    return np.ascontiguousarray(
        wT.reshape(Dd // 128, 128, Fd).transpose(1, 0, 2).reshape(
            128, (Dd // 128) * Fd)).astype(NPF8)


def make_in_maps(queries, keys, values, query_mask, key_mask,
                 Wg, bg, Wv, bv, Wq, bq, Wk, bk, Wo, bo):
    B, T_full, _ = queries.shape
    Th = T_full // 2
    NT = Th // 128
    f32 = np.float32
    km_f = np.asarray(key_mask, f32)
    # key mask folded into key operand + its bias row
    kM = np.asarray(keys, f32) * km_f[:, :, None]
    qTb = np.ascontiguousarray(np.asarray(queries, f32).transpose(0, 2, 1))
    kTb = np.ascontiguousarray(kM.transpose(0, 2, 1))
    vb = np.asarray(values, f32)
    shared = {
        "wkc": _chunk_W_aug(np.ascontiguousarray(np.asarray(Wk, f32).T), bk),
        "wqc": _chunk_W_aug(np.ascontiguousarray(np.asarray(Wq, f32).T), bq),
        "wgc": _chunk_W(np.ascontiguousarray(np.asarray(Wg, f32).T)),
        # fold operand: rhs[d_raw, i] = Wv[i, d_raw] -> chunk Wv.T over d_raw
        "wvc": _chunk_W_aug(np.ascontiguousarray(np.asarray(Wv, f32).T), bv),
        "woc": _chunk_W(np.ascontiguousarray(np.asarray(Wo, f32).T)),
        "bgc": np.ascontiguousarray(
            np.asarray(bg, f32).reshape(D // 128, 128).T),
    }
    in_maps = []
    for c in range(N_CORES):
        b, h = divmod(c, 2)
        sl = slice(h * Th, (h + 1) * Th)
        m = dict(shared)
        m["qTc"] = _chunk_T_aug(qTb[b][:, sl], 512,
                                extra_row=np.ones(Th, f32))
        m["kTc"] = _chunk_T_aug(kTb[b][:, sl], 128, extra_row=km_f[b, sl])
        vt = np.zeros((NT, 128, D + 128), dtype=NPF8)
        vt[:, :, :D] = vb[b, sl].reshape(NT, 128, D).astype(NPF8)
        vt[:, :, D] = 1.0
        m["vTc"] = vt
        m["qm"] = np.ascontiguousarray(
            np.asarray(query_mask[b, sl], f32).reshape(NT, 128).T)
        in_maps.append(m)
    return in_maps


def kernel(queries, keys, values, query_mask, key_mask,
           Wg, bg, Wv, bv, Wq, bq, Wk, bk, Wo, bo, _trace=False):
    B, T_full, _ = queries.shape
    Th = T_full // 2
    nc = _get_nc(Th)
    in_maps = make_in_maps(queries, keys, values, query_mask, key_mask,
                           Wg, bg, Wv, bv, Wq, bq, Wk, bk, Wo, bo)
    res = run_bass_kernel_spmd(nc, in_maps, core_ids=list(range(N_CORES)),
                               trace=_trace)
    out = np.empty((B, T_full, D), np.float32)
    for c in range(N_CORES):
        b, h = divmod(c, 2)
        out[b, h * Th:(h + 1) * Th] = res.results[c]["out"].astype(np.float32)
    # host-side epilogue: + qm * bo
    out += (np.asarray(query_mask, np.float32)[:, :, None]
            * np.asarray(bo, np.float32)[None, None, :])
    if _trace:
        kernel._last_res = res
    return out


# revision 18
# speedup vs baseline: 1.1713x; 1.1713x over previous
"""GAU (gated attention unit, relu^2 linear attention) Trainium2 kernel, v2.

Sharding: 8 cores = batch (4) x T-half (2).  Each core handles 2048 rows
of both the query and key/value streams of one batch.  The kv/k_sum
reduction over keys is completed with a 2-rank AllReduce between the two
cores of each batch (bf16 payload); everything else is fully local.

v2 structure (vs v1): the value projection is folded to AFTER the key
reduction -- kv_raw[e,s] = sum_k values[k,e]*kfeat[k,s] is accumulated
against RAW values (token-major), allreduced, then kv = Wv @ kv_raw +
bv x k_sum is a tiny [1024x512 @ 1024x1024] fold done per-core.  This
removes the per-token v projection (Tk*D*D MACs).  k_sum rides along as
an extra matmul with an all-ones stationary vector.  All biases (bk,
bq, bv) are folded into matmul contraction rows via padded chunks; the
key mask is folded into the key operand on the host; the query mask and
output bias bo are applied on the host after the gather.  relu^2 is a
single vector stt (max(x,0)*x).  Matmuls are fp8(e4m3) DoubleRow
throughout; PSUM accumulation fp32.  Output is bf16, upcast on host.

Scales: kvr_sb = kv_raw/32, kv_sb = kv/32, ks_sb = k_sum/8,
uq = 0.5*u*(qkv/32) -> zi = qm*64/(8*zp + EPS).
"""
import sys

sys.path.insert(0, "/opt/trn_rl_repo")

import ml_dtypes
import numpy as np
import concourse.bass as bass
import concourse.mybir as mybir
import concourse.tile as tile
from concourse.bass_utils import run_bass_kernel_spmd

AF = mybir.ActivationFunctionType
ALU = mybir.AluOpType
PM = mybir.MatmulPerfMode
F32 = mybir.dt.float32
BF16 = mybir.dt.bfloat16
F8 = mybir.dt.float8e4
NPBF = ml_dtypes.bfloat16
NPF8 = mybir.dt.np(F8)

N_CORES = 8
D = 1024
S = 512
EPS = 1e-6
KVR = 32.0   # kvr_sb/kv_sb hold kv_raw/32, kv/32
KS = 8.0     # ks_sb holds k_sum/8
UQ = 0.5     # uq holds 0.5 * u * (qkv/32)


def split_sync_waits(nc, max_waits=1):
    """The pinned walrus accepts at most one sync wait per instruction;
    hoist excess waits onto same-engine NoOps inserted before the
    offending instruction (same engine => identical semantics)."""
    n = 0
    for bb in nc.main_func.blocks:
        out = []
        for inst in bb.instructions:
            si = inst.sync_info
            if si is not None and si.on_wait and len(si.on_wait) > max_waits:
                waits = list(si.on_wait)
                spill, keep = waits[:-max_waits], waits[-max_waits:]
                for j in range(0, len(spill), max_waits):
                    nop = mybir.InstNoOp(
                        name=f"{inst.name}_wsp{j}",
                        engine=inst.engine,
                        ins=[],
                        outs=[],
                        bass_nofuse=True,
                        sync_info=mybir.SyncInfo(
                            on_wait=spill[j : j + max_waits], on_update=[]
                        ),
                    )
                    nc.register_instruction(nop)
                    out.append(nop)
                    n += 1
                si.on_wait[:] = keep
            out.append(inst)
        bb.instructions[:] = out
    return n


def dedup_ldweights(nc):
    """Delete an InstLdweights whose weights AP is byte-identical to the
    previous PE ldweights with only matmuls in between (the PE array
    keeps its stationary operand across matmuls).  Waits/updates of the
    deleted LDW migrate to the next PE instruction (its paired matmul)
    so sync semantics are preserved.  Run BEFORE split_sync_waits."""
    n = 0
    for bb in nc.main_func.blocks:
        out = []
        last_sig = None
        pending = None  # sync carried from a deleted LDW
        for inst in bb.instructions:
            tn = type(inst).__name__
            if getattr(inst, "engine", None) != mybir.EngineType.PE:
                out.append(inst)
                continue
            if tn == "InstLdweights":
                ap = inst.ins[0]
                sig = (ap.memref, ap.offset, tuple(map(tuple, ap.ap)),
                       str(inst.perf_mode), str(inst.is_transpose),
                       str(inst.tile_position), str(inst.tile_size))
                if sig == last_sig:
                    si = inst.sync_info
                    if si is not None and (si.on_wait or si.on_update):
                        if pending is None:
                            pending = ([], [])
                        pending[0].extend(si.on_wait)
                        pending[1].extend(si.on_update)
                    n += 1
                    continue
                last_sig = sig
            elif tn == "InstMatmult":
                if inst.is_transpose:
                    last_sig = None
            else:
                # any other PE instruction: be conservative
                last_sig = None
            if pending is not None:
                si = inst.sync_info
                if si is None:
                    inst.sync_info = mybir.SyncInfo(
                        on_wait=list(pending[0]), on_update=list(pending[1]))
                else:
                    si.on_wait[:] = list(pending[0]) + list(si.on_wait)
                    si.on_update[:] = list(si.on_update) + list(pending[1])
                pending = None
            out.append(inst)
        assert pending is None
        bb.instructions[:] = out
    return n


def build_nc(T=2048, use_collective=True):
    NT = T // 128    # t-tiles
    NQ = T // 512    # q-chunks (phase 2)
    NDA = 5          # DoubleRow pairs over d (4 data + 1 bias/pad)
    NS = S // 128    # s-tiles
    NF = D // 128    # f-tiles (gate dim)

    nc = bass.Bass("TRN2", target_bir_lowering=False, debug=False,
                   num_devices=N_CORES)

    # ---- I/O ------------------------------------------------------------
    # host-prepped: chunked, fp8, masks/biases folded in (see make_in_maps)
    qTc = nc.dram_tensor("qTc", [NQ, 128, NDA * 2 * 512], F8, kind="ExternalInput")
    kTc = nc.dram_tensor("kTc", [NT, 128, NDA * 2 * 128], F8, kind="ExternalInput")
    vTc = nc.dram_tensor("vTc", [NT, 128, D + 128], F8, kind="ExternalInput")
    wkc = nc.dram_tensor("wkc", [128, NDA * 2 * S], F8, kind="ExternalInput")
    wqc = nc.dram_tensor("wqc", [128, NDA * 2 * S], F8, kind="ExternalInput")
    wgc = nc.dram_tensor("wgc", [128, (D // 128) * D], F8, kind="ExternalInput")
    wvc = nc.dram_tensor("wvc", [128, NDA * 2 * D], F8, kind="ExternalInput")
    woc = nc.dram_tensor("woc", [128, (D // 128) * D], F8, kind="ExternalInput")
    bgc = nc.dram_tensor("bgc", [128, NF], F32, kind="ExternalInput")
    qm = nc.dram_tensor("qm", [128, NT], F32, kind="ExternalInput")
    out = nc.dram_tensor("out", [T, D], BF16, kind="ExternalOutput")

    with tile.TileContext(nc) as tc:
        with tc.tile_pool(name="const", bufs=1) as cp:
            # first-needed first
            wk_sb = cp.tile([128, NDA * 2, S], F8)
            nc.sync.dma_start(wk_sb[:], wkc.ap())
            # persistent data
            v_all = cp.tile([128, NT, D + 128], F8)
            k_nat = cp.tile([128, NT, S], F8)
            q_sb = cp.tile([128, NS, T], F8)
            u_sb = cp.tile([128, NF, T], F8)
            uq_f8 = cp.tile([128, NF, T], F8)
            kvr_sb = cp.tile([128, NDA * 2, S], F8)
            kv_sb = cp.tile([128, NS, D], F8)
            ks_sb = cp.tile([128, NS, 1], F8)
            # deferred-load tiles
            wq_sb = cp.tile([128, NDA * 2, S], F8)
            wg_sb = cp.tile([128, D // 128, D], F8)
            wv_sb = cp.tile([128, NDA * 2, D], F8)
            wo_sb = cp.tile([128, D // 128, D], F8)
            qm_sb = cp.tile([128, NT], F32)
            bg_sb = cp.tile([128, NF], F32)
            eps_sb = cp.tile([128, NQ], F32)

            with tc.tile_pool(name="dram", bufs=1, space="DRAM") as dram, \
                 tc.tile_pool(name="pf", bufs=1) as pf:
                bounce_in = dram.tile([2 * S + 1, S], BF16)
                bounce_out = dram.tile([2 * S + 1, S], BF16)

                def load_qc(qch):
                    qc = pf.tile([128, NDA * 2, 512], F8, name="qc",
                                 tag="qc", bufs=4)
                    nc.sync.dma_start(qc[:], qTc.ap()[qch])
                    return qc
                qc_pre = {}

                # ============ phase 1: k feats, kv_raw e0..3, k_sum ======
                with tc.tile_pool(name="p1", bufs=1) as p1, \
                     tc.tile_pool(name="ps1", bufs=1, space="PSUM") as ps1:
                    kv04 = [ps1.tile([128, S], F32, name=f"kv0_{e}", tag="kv0",
                                     bufs=NS) for e in range(NS)]

                    for j in range(NT // 2):
                        for tt in (2 * j, 2 * j + 1):
                            kc = p1.tile([128, NDA * 2, 128], F8, name="kc",
                                         tag="kc", bufs=4)
                            nc.sync.dma_start(kc[:], kTc.ap()[tt])
                            nc.sync.dma_start(v_all[:, tt, :], vTc.ap()[tt])
                            if tt == 0:
                                # zero the bias/pad chunks of kvr (stale f8
                                # NaN x 0 would poison the fold PSUM)
                                nc.gpsimd.memset(kvr_sb[:, 8:10, :], 0.0)
                            if tt == 1:
                                qc_pre[0] = load_qc(0)
                            if tt == 11:
                                qc_pre[1] = load_qc(1)
                                nc.sync.dma_start(wq_sb[:], wqc.ap())
                                nc.sync.dma_start(qm_sb[:], qm.ap())
                                nc.sync.dma_start(bg_sb[:], bgc.ap())
                                nc.vector.memset(eps_sb[:], EPS)
                            if tt == 2:
                                nc.sync.dma_start(wg_sb[:], wgc.ap())
                            if tt == 3:
                                nc.sync.dma_start(wv_sb[:], wvc.ap())
                            if tt == 4:
                                nc.sync.dma_start(wo_sb[:], woc.ap())

                            # k feats: relu(K Wk^T + bk)^2 * km (bias+mask
                            # folded into padded chunks) -> k_nat[:,tt,:]
                            kb = ps1.tile([128, S], F32, name="kb", tag="kb",
                                          bufs=3)
                            for c in range(NDA):
                                nc.tensor.matmul(kb[:], kc[:, 2 * c:2 * c + 2, :],
                                                 wk_sb[:, 2 * c:2 * c + 2, :],
                                                 start=(c == 0), stop=(c == NDA - 1),
                                                 perf_mode=PM.DoubleRow)
                            krelu = p1.tile([128, S], BF16, name="krelu",
                                            tag="krelu", bufs=2)
                            nc.vector.tensor_scalar_max(krelu[:], kb[:], 0.0)
                            nc.scalar.activation(k_nat[:, tt, :], krelu[:],
                                                 AF.Square)

                        # kv_raw e-blocks 0..3 accumulate over t pairs
                        for e in range(NS):
                            nc.tensor.matmul(
                                kv04[e][:],
                                v_all[:, 2 * j:2 * j + 2, e * 128:(e + 1) * 128],
                                k_nat[:, 2 * j:2 * j + 2, :],
                                start=(j == 0), stop=(j == NT // 2 - 1),
                                perf_mode=PM.DoubleRow)

                    for e in range(NS):
                        kvst = p1.tile([128, S], BF16, name="kvst",
                                       tag="kvst", bufs=2)
                        nc.scalar.activation(kvst[:], kv04[e][:], AF.Copy)
                        nc.sync.dma_start(
                            bounce_in[e * 128:(e + 1) * 128, :], kvst[:])
                    if use_collective:
                        nc.gpsimd.collective_compute(
                            "AllReduce", ALU.add,
                            replica_groups=[[0, 1], [2, 3], [4, 5], [6, 7]],
                            ins=[bounce_in[0:S, :].opt()],
                            outs=[bounce_out[0:S, :].opt()])

                # ============ phase 1b: kv_raw e4..7 =====================
                with tc.tile_pool(name="p1b", bufs=1) as p1b, \
                     tc.tile_pool(name="ps1b", bufs=1, space="PSUM") as ps1b:
                    kv48 = [ps1b.tile([128, S], F32, name=f"kv1_{e}", tag="kv1",
                                      bufs=NS) for e in range(NS)]
                    ks_blk = ps1b.tile([128, S], F32, name="ks_blk",
                                       tag="ksb", bufs=1)
                    for j in range(NT // 2):
                        for e in range(NS):
                            nc.tensor.matmul(
                                kv48[e][:],
                                v_all[:, 2 * j:2 * j + 2,
                                      S + e * 128:S + (e + 1) * 128],
                                k_nat[:, 2 * j:2 * j + 2, :],
                                start=(j == 0), stop=(j == NT // 2 - 1),
                                perf_mode=PM.DoubleRow)
                        nc.tensor.matmul(
                            ks_blk[:],
                            v_all[:, 2 * j:2 * j + 2, D:D + 128],
                            k_nat[:, 2 * j:2 * j + 2, :],
                            start=(j == 0), stop=(j == NT // 2 - 1),
                            perf_mode=PM.DoubleRow)
                    for e in range(NS):
                        kvst1 = p1b.tile([128, S], BF16, name="kvst1",
                                         tag="kvst1", bufs=2)
                        nc.scalar.activation(kvst1[:], kv48[e][:], AF.Copy)
                        nc.sync.dma_start(
                            bounce_in[S + e * 128:S + (e + 1) * 128, :],
                            kvst1[:])
                    ksst = p1b.tile([1, S], BF16, name="ksst")
                    nc.scalar.activation(ksst[:], ks_blk[0:1, :], AF.Copy)
                    nc.sync.dma_start(bounce_in[2 * S:2 * S + 1, :], ksst[:])


                with tc.tile_pool(name="p2", bufs=1) as p2, \
                     tc.tile_pool(name="ps2a", bufs=1, space="PSUM") as ps2a:
                    if use_collective:
                        nc.gpsimd.collective_compute(
                            "AllReduce", ALU.add,
                            replica_groups=[[0, 1], [2, 3], [4, 5], [6, 7]],
                            ins=[bounce_in[S:2 * S + 1, :].opt()],
                            outs=[bounce_out[S:2 * S + 1, :].opt()])
                        kv_src = bounce_out
                    else:
                        kv_src = bounce_in

                    # ---- pass A: q feats + u gate (no kv dependency) ----
                    # two q-chunk halves: stationary shared across the two
                    # chunks of a half; second half's DMAs issue at the
                    # start of the first (keeps q traffic out of phase 1)
                    for qlist in ((0, 1), (2, 3)):
                        for qch in qlist:
                            if qch not in qc_pre:
                                qc_pre[qch] = load_qc(qch)
                        if qlist[0] == 0:
                            for qch in (2, 3):
                                qc_pre[qch] = load_qc(qch)
                        qcs = {qch: qc_pre[qch] for qch in qlist}
                        for s in range(NS):
                            qfs = {q: ps2a.tile([128, 512], F32, name=f"qf{q}",
                                                tag="mm", bufs=8)
                                   for q in qlist}
                            for c in range(NDA):
                                for qch in qlist:
                                    nc.tensor.matmul(
                                        qfs[qch][:],
                                        wq_sb[:, 2 * c:2 * c + 2,
                                              s * 128:(s + 1) * 128],
                                        qcs[qch][:, 2 * c:2 * c + 2, :],
                                        start=(c == 0), stop=(c == NDA - 1),
                                        perf_mode=PM.DoubleRow)
                            for qch in qlist:
                                t0 = qch * 512
                                qrelu = p2.tile([128, 512], BF16,
                                                name="qrelu", tag="qrelu",
                                                bufs=4)
                                nc.vector.tensor_scalar_max(qrelu[:],
                                                            qfs[qch][:], 0.0)
                                nc.vector.tensor_mul(q_sb[:, s, t0:t0 + 512],
                                                     qrelu[:], qrelu[:])
                        for f in range(NF):
                            ufs = {q: ps2a.tile([128, 512], F32, name=f"uf{q}",
                                                tag="mm", bufs=8)
                                   for q in qlist}
                            for c in range(NDA - 1):
                                for qch in qlist:
                                    nc.tensor.matmul(
                                        ufs[qch][:],
                                        wg_sb[:, 2 * c:2 * c + 2,
                                              f * 128:(f + 1) * 128],
                                        qcs[qch][:, 2 * c:2 * c + 2, :],
                                        start=(c == 0), stop=(c == NDA - 2),
                                        perf_mode=PM.DoubleRow)
                            for qch in qlist:
                                t0 = qch * 512
                                nc.scalar.activation(u_sb[:, f, t0:t0 + 512],
                                                     ufs[qch][:], AF.Silu,
                                                     bias=bg_sb[:, f:f + 1])

                    # ---- unpack kv_raw + k_sum from the collective ------
                    kvf_sb = p2.tile([128, 2 * NS, S], BF16, name="kvf")
                    nc.sync.dma_start(
                        kvf_sb[:],
                        kv_src[0:2 * S, :].rearrange("(c p) s -> p c s", p=128))
                    ksf = p2.tile([1, S], BF16, name="ksf")
                    nc.sync.dma_start(ksf[:], kv_src[2 * S:2 * S + 1, :])
                    ks4 = p2.tile([128, NS], BF16, name="ks4")
                    nc.sync.dma_start(
                        ks4[:],
                        kv_src[2 * S:2 * S + 1, :].rearrange(
                            "o (c p) -> p (c o)", p=128))
                    for c in range(NS):
                        nc.scalar.activation(kvr_sb[:, 2 * c:2 * c + 2, :],
                                             kvf_sb[:, 2 * c:2 * c + 2, :],
                                             AF.Copy, scale=1.0 / KVR)
                    nc.scalar.activation(kvr_sb[0:1, 8, :], ksf[:], AF.Copy,
                                         scale=1.0 / KVR)
                    nc.vector.tensor_scalar_mul(ks_sb[:, :, 0], ks4[:],
                                                1.0 / KS)

                with tc.tile_pool(name="p2b", bufs=1) as p2b, \
                     tc.tile_pool(name="ps2b", bufs=1, space="PSUM") as ps2b:
                    # ---- fold kv = Wv @ kv_raw + bv x k_sum -------------
                    for sb in range(NS):
                        fps = [ps2b.tile([128, 512], F32, name=f"fp{h}",
                                         tag="mm", bufs=4) for h in range(2)]
                        for c in range(NDA):
                            for half in range(2):
                                nc.tensor.matmul(
                                    fps[half][:],
                                    kvr_sb[:, 2 * c:2 * c + 2,
                                           sb * 128:(sb + 1) * 128],
                                    wv_sb[:, 2 * c:2 * c + 2,
                                          half * 512:(half + 1) * 512],
                                    start=(c == 0), stop=(c == NDA - 1),
                                    perf_mode=PM.DoubleRow)
                        for half in range(2):
                            nc.vector.tensor_copy(
                                kv_sb[:, sb, half * 512:(half + 1) * 512],
                                fps[half][:])

                    # ---- pass B: qkv (f-major), z, output projection ----
                    for f in range(NF):
                        qks = [ps2b.tile([128, 512], F32, name=f"qk{q}",
                                         tag="mm", bufs=4) for q in range(NQ)]
                        for c in range(NS // 2):
                            for qch in range(NQ):
                                nc.tensor.matmul(
                                    qks[qch][:],
                                    kv_sb[:, 2 * c:2 * c + 2,
                                          f * 128:(f + 1) * 128],
                                    q_sb[:, 2 * c:2 * c + 2,
                                         qch * 512:(qch + 1) * 512],
                                    start=(c == 0), stop=(c == NS // 2 - 1),
                                    perf_mode=PM.DoubleRow)
                        for qch in range(NQ):
                            t0 = qch * 512
                            nc.vector.scalar_tensor_tensor(
                                uq_f8[:, f, t0:t0 + 512],
                                u_sb[:, f, t0:t0 + 512], UQ, qks[qch][:],
                                op0=ALU.mult, op1=ALU.mult)

                    for qch in range(NQ):
                        zp = ps2b.tile([128, 512], F32, name="zp", tag="mm",
                                       bufs=4)
                        for tt in range(4):
                            ti = qch * 4 + tt
                            for c in range(NS // 2):
                                nc.tensor.matmul(
                                    zp[:, tt:tt + 1],
                                    q_sb[:, 2 * c:2 * c + 2,
                                         ti * 128:(ti + 1) * 128],
                                    ks_sb[:, 2 * c:2 * c + 2, :],
                                    start=(c == 0), stop=(c == NS // 2 - 1),
                                    perf_mode=PM.DoubleRow)
                        z1 = p2b.tile([128, NQ], F32, name="z1", tag="z1",
                                      bufs=2)
                        nc.vector.scalar_tensor_tensor(
                            z1[:], zp[:, 0:NQ], KS, eps_sb[:], op0=ALU.mult,
                            op1=ALU.add)
                        z2 = p2b.tile([128, NQ], F32, name="z2", tag="z2",
                                      bufs=2)
                        nc.vector.reciprocal(z2[:], z1[:])
                        zi4 = p2b.tile([128, NQ], F32, name="zi4", tag="zi4",
                                       bufs=2)
                        nc.vector.scalar_tensor_tensor(
                            zi4[:], qm_sb[:, qch * 4:(qch + 1) * 4],
                            KVR / UQ, z2[:], op0=ALU.mult, op1=ALU.mult)

                        for tt in range(4):
                            ti = qch * 4 + tt
                            op = ps2b.tile([128, D], F32, name="op",
                                           tag="out", bufs=2)
                            for f2 in range(NF // 2):
                                for half in range(2):
                                    nc.tensor.matmul(
                                        op[:, half * 512:(half + 1) * 512],
                                        uq_f8[:, 2 * f2:2 * f2 + 2,
                                              ti * 128:(ti + 1) * 128],
                                        wo_sb[:, 2 * f2:2 * f2 + 2,
                                              half * 512:(half + 1) * 512],
                                        start=(f2 == 0),
                                        stop=(f2 == NF // 2 - 1),
                                        perf_mode=PM.DoubleRow)
                            o_sb = p2b.tile([128, D], BF16, name="o_sb",
                                            tag="o_sb", bufs=3)
                            nc.scalar.activation(o_sb[:], op[:], AF.Copy,
                                                 scale=zi4[:, tt:tt + 1])
                            nc.sync.dma_start(
                                out.ap()[ti * 128:(ti + 1) * 128, :], o_sb[:])

    dedup_ldweights(nc)
    split_sync_waits(nc)
    return nc


_NC_CACHE = {}


def _get_nc(T, use_collective=True):
    key = (T, use_collective)
    if key not in _NC_CACHE:
        _NC_CACHE[key] = build_nc(T, use_collective)
    return _NC_CACHE[key]


def _chunk_T_aug(xT, chunk, extra_row=None):
    """[D, T] -> [T//chunk, 128, 10*chunk] fp8: chunks 0..7 d-major data,
    chunk 8 row 0 = extra_row (per-token), chunk 9 = 0."""
    Dd, T = xT.shape
    nt = T // chunk
    out = np.zeros((nt, 128, 10 * chunk), dtype=NPF8)
    x = xT.reshape(Dd // 128, 128, nt, chunk).transpose(2, 1, 0, 3)
    out[:, :, : (Dd // 128) * chunk] = x.reshape(nt, 128, -1).astype(NPF8)
    if extra_row is not None:
        out[:, 0, 8 * chunk:9 * chunk] = extra_row.reshape(nt, chunk).astype(NPF8)
    return np.ascontiguousarray(out)


def _chunk_W_aug(wT, extra_row=None):
    """[D, F] -> [128, 10*F] fp8: chunks 0..7 d-major, chunk 8 row 0 =
    extra_row (bias over F), chunk 9 = 0."""
    Dd, Fd = wT.shape
    out = np.zeros((128, 10 * Fd), dtype=NPF8)
    w = wT.reshape(Dd // 128, 128, Fd).transpose(1, 0, 2).reshape(128, -1)
    out[:, : (Dd // 128) * Fd] = w.astype(NPF8)
    if extra_row is not None:
        out[0, 8 * Fd:9 * Fd] = np.asarray(extra_row, np.float32).astype(NPF8)
    return np.ascontiguousarray(out)


def _chunk_W(wT):
    Dd, Fd = wT.shape
    return np.ascontiguousarray(
        wT.reshape(Dd // 128, 128, Fd).transpose(1, 0, 2).reshape(
            128, (Dd // 128) * Fd)).astype(NPF8)


def make_in_maps(queries, keys, values, query_mask, key_mask,
                 Wg, bg, Wv, bv, Wq, bq, Wk, bk, Wo, bo):
    B, T_full, _ = queries.shape
    Th = T_full // 2
    NT = Th // 128
    f32 = np.float32
    km_f = np.asarray(key_mask, f32)
    # key mask folded into key operand + its bias row
    kM = np.asarray(keys, f32) * km_f[:, :, None]
    qTb = np.ascontiguousarray(np.asarray(queries, f32).transpose(0, 2, 1))
    kTb = np.ascontiguousarray(kM.transpose(0, 2, 1))
    vb = np.asarray(values, f32)
    shared = {
        "wkc": _chunk_W_aug(np.ascontiguousarray(np.asarray(Wk, f32).T), bk),
        "wqc": _chunk_W_aug(np.ascontiguousarray(np.asarray(Wq, f32).T), bq),
        "wgc": _chunk_W(np.ascontiguousarray(np.asarray(Wg, f32).T)),
        # fold operand: rhs[d_raw, i] = Wv[i, d_raw] -> chunk Wv.T over d_raw
        "wvc": _chunk_W_aug(np.ascontiguousarray(np.asarray(Wv, f32).T), bv),
        "woc": _chunk_W(np.ascontiguousarray(np.asarray(Wo, f32).T)),
        "bgc": np.ascontiguousarray(
            np.asarray(bg, f32).reshape(D // 128, 128).T),
    }
    in_maps = []
    for c in range(N_CORES):
        b, h = divmod(c, 2)
        sl = slice(h * Th, (h + 1) * Th)
        m = dict(shared)
        m["qTc"] = _chunk_T_aug(qTb[b][:, sl], 512,
                                extra_row=np.ones(Th, f32))
        m["kTc"] = _chunk_T_aug(kTb[b][:, sl], 128, extra_row=km_f[b, sl])
        vt = np.zeros((NT, 128, D + 128), dtype=NPF8)
        vt[:, :, :D] = vb[b, sl].reshape(NT, 128, D).astype(NPF8)
        vt[:, :, D] = 1.0
        m["vTc"] = vt
        m["qm"] = np.ascontiguousarray(
            np.asarray(query_mask[b, sl], f32).reshape(NT, 128).T)
        in_maps.append(m)
    return in_maps


def kernel(queries, keys, values, query_mask, key_mask,
           Wg, bg, Wv, bv, Wq, bq, Wk, bk, Wo, bo, _trace=False):
    B, T_full, _ = queries.shape
    Th = T_full // 2
    nc = _get_nc(Th)
    in_maps = make_in_maps(queries, keys, values, query_mask, key_mask,
                           Wg, bg, Wv, bv, Wq, bq, Wk, bk, Wo, bo)
    res = run_bass_kernel_spmd(nc, in_maps, core_ids=list(range(N_CORES)),
                               trace=_trace)
    out = np.empty((B, T_full, D), np.float32)
    for c in range(N_CORES):
        b, h = divmod(c, 2)
        out[b, h * Th:(h + 1) * Th] = res.results[c]["out"].astype(np.float32)
    # host-side epilogue: + qm * bo
    out += (np.asarray(query_mask, np.float32)[:, :, None]
            * np.asarray(bo, np.float32)[None, None, :])
    if _trace:
        kernel._last_res = res
    return out
